# revision 3
# baseline (speedup 1.0000x reference)
"""v2 Trainium2 Bass kernel for nn_DeformAtten1D.

Changes vs baseline:
- xT fed pre-transposed from host (contiguous loads, no rearranged-AP DMA)
- qT via on-chip tensor transposes of gq (drops the duplicate q projection
  and the qt_scr DRAM round trip)
- offset network collapsed to a single 1-channel conv (W' = w2^T @ W1),
  with boundary correction for the conv2 zero-padding
- x_sampled kept in SBUF via on-chip transposes (drops xs_scr round trip)
- output projection x-stationary so stores are contiguous [l, o]
"""
import sys
for _p in ('/opt/trn_rl_repo', '/root/.axon_site/_ro/trn_rl_repo'):
    if _p not in sys.path:
        sys.path.insert(0, _p)

import numpy as np
import ml_dtypes

import concourse.bass as bass
import concourse.bacc as bacc
import concourse.mybir as mybir
import concourse.tile as tile
from concourse.masks import make_identity
from concourse.library_config import mlp

F32 = mybir.dt.float32
BF16 = mybir.dt.bfloat16
I32 = mybir.dt.int32
I16 = mybir.dt.int16
AF = mybir.ActivationFunctionType
OP = mybir.AluOpType
BF = ml_dtypes.bfloat16


class Cfg:
    def set_mm_gather(self, on=True):
        self.mm_gather = on
        if on:
            self.GO, self.GR = 9, 32
        else:
            self.GO, self.GR = 6, 12

    def __init__(self, B_SH, L, C, H, G, K=5):
        self.B_SH, self.L, self.C, self.H, self.G, self.K = B_SH, L, C, H, G, K
        self.GC = C // G
        self.DH = C // H
        assert self.DH == 64
        self.PAD = K // 2
        self.L4 = L + 2 * self.PAD
        self.sn = L / (self.L4 - 1)
        self.KT = C // 128
        self.NS = L // 128
        self.SLG = self.GC // 128
        self.NB = L // 512
        self.NO = C // 512
        self.MH = self.KT // self.NO   # m-tiles per weight half-block
        self.GR = 12
        self.GO = 6
        if getattr(self, 'mm_gather', False):
            self.GO, self.GR = 9, 32
        self.P2 = H // 2
        assert self.P2 == self.KT
        assert L % 512 == 0 and C % 512 == 0 and self.GC % 128 == 0


def declare(nc, cfg):
    c = cfg
    t = {}
    t['xgbf'] = nc.dram_tensor("xgbf", [c.B_SH * (c.L + c.GR), c.C], BF16, kind="ExternalInput")
    t['xtbf'] = nc.dram_tensor("xtbf", [c.B_SH * c.C, c.L], BF16, kind="ExternalInput")
    t['rtab'] = nc.dram_tensor("rtab", [c.C, c.L], BF16, kind="ExternalInput")
    t['wpr'] = nc.dram_tensor("wpr", [c.GC, c.K], BF16, kind="ExternalInput")
    for nm in ('wqT', 'wkT', 'wvT', 'woutT'):
        t[nm] = nc.dram_tensor(nm, [c.C, c.C], BF16, kind="ExternalInput")
    t['bq_col'] = nc.dram_tensor("bq_col", [c.C, 1], F32, kind="ExternalInput")
    t['bk_row'] = nc.dram_tensor("bk_row", [1, c.C], BF16, kind="ExternalInput")
    t['bo_row'] = nc.dram_tensor("bo_row", [1, c.C], BF16, kind="ExternalInput")
    t['bcompc'] = nc.dram_tensor("bcompc", [1, 1], F32, kind="ExternalInput")
    t['w2b1c'] = nc.dram_tensor("w2b1c", [1, 1], F32, kind="ExternalInput")
    t['posg'] = nc.dram_tensor("posg", [128, c.NS], F32, kind="ExternalInput")
    if getattr(c, 'mm_gather', False):
        t['iotaR'] = nc.dram_tensor("iotaR", [128, 160], F32, kind="ExternalInput")
        t['sub2'] = nc.dram_tensor("sub2", [128, c.NS], F32, kind="ExternalInput")
    t['out2d'] = nc.dram_tensor("out2d", [c.B_SH * c.L, c.C], F32, kind="ExternalOutput")
    dbg = getattr(c, 'debug', False)
    if dbg:
        t['d_gq'] = nc.dram_tensor("d_gq", [128, c.KT, c.L + 4], BF16, kind="ExternalOutput")
        t['d_offrow'] = nc.dram_tensor("d_offrow", [c.G, c.L], F32, kind="ExternalOutput")
        t['d_psm'] = nc.dram_tensor("d_psm", [c.G, 128, c.NS], F32, kind="ExternalOutput")
        t['d_w1'] = nc.dram_tensor("d_w1", [c.G, 128, c.NS], F32, kind="ExternalOutput")
        t['d_xs'] = nc.dram_tensor("d_xs", [128, c.KT, c.L], BF16, kind="ExternalOutput")
        t['d_kT'] = nc.dram_tensor("d_kT", [128, c.NS, c.C], BF16, kind="ExternalOutput")
        t['d_qsl'] = nc.dram_tensor("d_qsl", [c.P2, 128, c.NS, 128], BF16, kind="ExternalOutput")
        t['d_ablk'] = nc.dram_tensor("d_ablk", [c.P2, 128, 128], F32, kind="ExternalOutput")
        t['d_v'] = nc.dram_tensor("d_v", [128, c.KT, c.L], BF16, kind="ExternalOutput")
        t['d_ao'] = nc.dram_tensor("d_ao", [128, c.KT, c.L], BF16, kind="ExternalOutput")
    return t


def build(tc, t, cfg, ctx):
    c = cfg
    nc = tc.nc
    L, C, KT, NS, NB, NO, GC, SLG, G, K, MH = (c.L, c.C, c.KT, c.NS, c.NB, c.NO,
                                               c.GC, c.SLG, c.G, c.K, c.MH)
    scale = C ** -0.5

    nc.gpsimd.load_library(mlp)

    konst = ctx.enter_context(tc.tile_pool(name="konst", bufs=1))
    big = ctx.enter_context(tc.tile_pool(name="big", bufs=1))
    wp = ctx.enter_context(tc.tile_pool(name="wp", bufs=2))
    med = ctx.enter_context(tc.tile_pool(name="med", bufs=2))
    sm = ctx.enter_context(tc.tile_pool(name="sm", bufs=2))
    rp = ctx.enter_context(tc.tile_pool(name="rp", bufs=2))
    yp = ctx.enter_context(tc.tile_pool(name="yp", bufs=3))
    qp = ctx.enter_context(tc.tile_pool(name="qp", bufs=1))
    orow = ctx.enter_context(tc.tile_pool(name="orow", bufs=3))
    spool = ctx.enter_context(tc.tile_pool(name="spool", bufs=3))
    dscr = ctx.enter_context(tc.tile_pool(name="dscr", bufs=2, space="DRAM"))
    psmm = ctx.enter_context(tc.tile_pool(name="psmm", bufs=3, space="PSUM"))
    pssc = ctx.enter_context(tc.tile_pool(name="pssc", bufs=1, space="PSUM"))
    pstr = ctx.enter_context(tc.tile_pool(name="pstr", bufs=3, space="PSUM"))
    psoff2 = ctx.enter_context(tc.tile_pool(name="psoff2", bufs=1, space="PSUM"))

    # ---- constants ----
    wpr_sb = konst.tile([128, SLG, K], BF16, tag="wpr")
    nc.sync.dma_start(out=wpr_sb[:], in_=bass.AP(
        tensor=t['wpr'].ap().tensor, offset=0,
        ap=[[K, 128], [128 * K, SLG], [1, K]]))
    posg_sb = konst.tile([128, NS], F32, tag="posg")
    nc.sync.dma_start(out=posg_sb[:], in_=t['posg'].ap())
    bq_col_sb = konst.tile([128, KT], F32, tag="bqc")
    nc.sync.dma_start(out=bq_col_sb[:], in_=bass.AP(
        tensor=t['bq_col'].ap().tensor, offset=0, ap=[[1, 128], [128, KT]]))
    bcomp_sb = konst.tile([128, 1], F32, tag="bco")
    nc.sync.dma_start(out=bcomp_sb[:], in_=bass.AP(
        tensor=t['bcompc'].ap().tensor, offset=0, ap=[[0, 128], [1, 1]]))
    w2b1_sb = konst.tile([G, 1], F32, tag="w2b1")
    nc.sync.dma_start(out=w2b1_sb[:], in_=bass.AP(
        tensor=t['w2b1c'].ap().tensor, offset=0, ap=[[0, G], [1, 1]]))
    bkr_sb = konst.tile([1, C], BF16, tag="bkr")
    nc.sync.dma_start(out=bkr_sb[:], in_=t['bk_row'].ap())
    bor_sb = konst.tile([1, C], BF16, tag="bor")
    nc.sync.dma_start(out=bor_sb[:], in_=t['bo_row'].ap())
    bkr_bc = konst.tile([128, C], BF16, tag="bkrb")
    nc.gpsimd.partition_broadcast(bkr_bc[:], bkr_sb[:])
    bor_bc = konst.tile([128, C], BF16, tag="borb")
    nc.gpsimd.partition_broadcast(bor_bc[:], bor_sb[:])
    ones_sb = konst.tile([1, 128], BF16, tag="ones")
    nc.vector.memset(ones_sb[:], 1.0)
    ones_col = konst.tile([G, 1], F32, tag="onesc")
    nc.vector.memset(ones_col[:], 1.0)
    onef_col = konst.tile([128, 1], F32, tag="onef")
    nc.vector.memset(onef_col[:], 1.0)
    zerof_col = konst.tile([128, 1], F32, tag="zerof")
    nc.vector.memset(zerof_col[:], 0.0)
    MMG = getattr(c, 'mm_gather', False)
    if MMG:
        iotaR_sb = konst.tile([128, 160], F32, tag="iotaR")
        nc.sync.dma_start(out=iotaR_sb[:], in_=t['iotaR'].ap())
        sub2_sb = konst.tile([128, NS], F32, tag="sub2")
        nc.sync.dma_start(out=sub2_sb[:], in_=t['sub2'].ap())
    ident = konst.tile([128, 128], F32, tag="ident")
    make_identity(nc, ident[:])
    identb = konst.tile([128, 128], BF16, tag="identb")
    make_identity(nc, identb[:])

    DBG = getattr(c, 'debug', False)
    NO_GATHER = getattr(c, 'no_gather', False)
    NO_CONV = getattr(c, 'no_conv', False)
    NO_QT = getattr(c, 'no_qt', False)
    NO_ATTN = getattr(c, 'no_attn', False)
    xg = t['xgbf'].ap()
    LG = L + c.GR

    def load_whalf(wname, hi):
        wh = wp.tile([128, KT, 512], BF16, tag="wblk")
        nc.sync.dma_start(out=wh[:], in_=bass.AP(
            tensor=t[wname].ap().tensor, offset=512 * hi,
            ap=[[C, 128], [128 * C, KT], [1, 512]]))
        return wh

    REPEAT = getattr(c, 'repeat', 1)
    for b in [bb for _ in range(REPEAT) for bb in range(c.B_SH)]:
        # ================= phase A: xT load (contiguous), q-pass =================
        xT = big.tile([128, KT, L], BF16, tag="bigX")
        for kt in range(KT):
            nc.sync.dma_start(
                out=xT[:, kt, :],
                in_=t['xtbf'].ap()[b * C + 128 * kt: b * C + 128 * (kt + 1), :])

        gq = big.tile([128, KT, L + 4], BF16, tag="bigGV")
        nc.gpsimd.memset(gq[:, :, 0:4], 0.0)
        for hi in range(NO):
            wq_h = load_whalf('wqT', hi)
            for mm_ in range(MH):
                m = hi * MH + mm_
                for n in range(NB):
                    ps = psmm.tile([128, 512], F32, tag="mm", space="PSUM")
                    for kt in range(KT):
                        nc.tensor.matmul(ps[:], lhsT=wq_h[:, kt, 128 * mm_:128 * (mm_ + 1)],
                                         rhs=xT[:, kt, 512 * n:512 * (n + 1)],
                                         start=(kt == 0), stop=(kt == KT - 1))
                    nc.scalar.activation(out=gq[:, m, 4 + 512 * n:4 + 512 * (n + 1)], in_=ps[:],
                                         func=AF.Identity, bias=bq_col_sb[:, m:m + 1], scale=1.0)

        if DBG and b == 0:
            nc.sync.dma_start(out=t['d_gq'].ap(), in_=gq[:])

        # ====== phase B0: composite offset conv -> off rows [G, L] ======
        # off[g, l'] = sum_{cin,t} W'[cin,t] * gq_pad[g*GC+cin, l'-4+t]  (+bias via tanh)
        # single-partition chunks (matmul base-partition rule), consumed
        # immediately into offp so only a small rotating row buffer is needed
        offp = psoff2.tile([128, G * NS], F32, tag="offp", space="PSUM")
        for g in range(G if not NO_CONV else 0):
            for n in range(NB):
                po = psmm.tile([128, 512], F32, tag="mm", space="PSUM")
                first = True
                for kt2 in range(SLG):
                    for tt_ in range(K):
                        nc.tensor.matmul(
                            po[0:1, :],
                            lhsT=wpr_sb[:, kt2, tt_:tt_ + 1],
                            rhs=gq[:, g * SLG + kt2, 512 * n + tt_: 512 * n + tt_ + 512],
                            start=first, stop=(kt2 == SLG - 1 and tt_ == K - 1))
                        first = False
                oc = orow.tile([1, 512], F32, tag="oc")
                nc.vector.tensor_copy(out=oc[0:1, :], in_=po[0:1, :])
                if n == 0:
                    # conv2 zero-pad boundary: at l' in {0,1} the inner conv
                    # reads zero-padding, so off = b2 there: overwrite the
                    # composite sum with -w2b1 (tanh bias adds bcomp back)
                    nc.vector.tensor_scalar(out=oc[0:1, 0:2], in0=oc[0:1, 0:2],
                                            scalar1=0.0, scalar2=None, op0=OP.mult)
                    nc.vector.tensor_scalar(out=oc[0:1, 0:2], in0=oc[0:1, 0:2],
                                            scalar1=w2b1_sb[0:1, :], scalar2=None,
                                            op0=OP.subtract)
                if DBG and b == 0:
                    nc.sync.dma_start(out=bass.AP(
                        tensor=t['d_offrow'].ap().tensor, offset=g * L + 512 * n,
                        ap=[[512, 1], [1, 512]]), in_=oc[0:1, :])
                # transpose chunk -> offp cols (col g*NS+s holds l = 128s+p)
                for k4 in range(4):
                    s = 4 * n + k4
                    nc.tensor.matmul(offp[:, g * NS + s:g * NS + s + 1],
                                     lhsT=oc[0:1, 128 * k4:128 * (k4 + 1)],
                                     rhs=ones_col[0:1, :],
                                     start=True, stop=True)

        if NO_CONV:
            nc.vector.memset(offp[:], 0.0)
        # ====== phase B1: qT transposes (independent of gather; fills tensor) ======
        qsls = []
        for pr in range(c.P2):
            qsl = qp.tile([128, NS, 128], BF16, tag=f"qsl{pr}")
            for lt in range(NS if not NO_QT else 0):
                trp = pstr.tile([128, 128], F32, tag="tr", space="PSUM")
                nc.tensor.transpose(trp[:], gq[:, pr, 4 + 128 * lt:4 + 128 * (lt + 1)], identb[:])
                nc.vector.tensor_copy(out=qsl[:, lt, :], in_=trp[:])
            qsls.append(qsl)
            if DBG and b == 0:
                nc.sync.dma_start(out=t['d_qsl'].ap()[pr], in_=qsl[:])

        # ====== phase B2: per group offsets -> sample -> xs ======
        if NO_GATHER:
            xs = xT
        else:
            xs = big.tile([128, KT, L], BF16, tag="bigX")
        if MMG and not NO_GATHER:
            # banded-matmul bilinear sampler: xs[c,l'] built directly via
            # psum = x_window[r,c].T @ S^T[r,l'], S[l,r] = -relu(1-|r-posrel|)
            x_lc = big.tile([128, NS + 1, C], BF16, tag="bigKA")
            for s in range(NS):
                nc.sync.dma_start(out=x_lc[:, s, :],
                                  in_=xg[b * LG + 128 * s: b * LG + 128 * (s + 1), :])
            nc.sync.dma_start(out=x_lc[0:32, NS, :],
                              in_=xg[b * LG + 128 * NS: b * LG + 128 * NS + 32, :])
            for g in range(G):
                tanh_t = sm.tile([128, NS], F32, tag="tanh")
                nc.scalar.activation(out=tanh_t[:], in_=offp[:, g * NS:(g + 1) * NS],
                                     func=AF.Tanh, bias=bcomp_sb[:, 0:1], scale=1.0)
                psm = sm.tile([128, NS], F32, tag="psm")
                nc.vector.tensor_scalar(out=psm[:], in0=tanh_t[:], scalar1=5.0 * c.sn,
                                        scalar2=None, op0=OP.mult)
                nc.vector.tensor_tensor(out=psm[:], in0=psm[:], in1=posg_sb[:], op=OP.add)
                if DBG and b == 0:
                    nc.sync.dma_start(out=t['d_psm'].ap()[g], in_=psm[:])
                prel = sm.tile([128, NS], F32, tag="prel")
                nc.vector.tensor_tensor(out=prel[:], in0=psm[:], in1=sub2_sb[:],
                                        op=OP.subtract)
                for s in range(NS):
                    # S = relu(1 - |d|), d = r - posrel  (hat weights, via ACT)
                    t1 = spool.tile([128, 160], F32, tag="t1")
                    nc.vector.tensor_scalar(out=t1[:], in0=iotaR_sb[:],
                                            scalar1=prel[:, s:s + 1], scalar2=None,
                                            op0=OP.subtract)
                    ta = spool.tile([128, 160], F32, tag="ta")
                    nc.scalar.activation(out=ta[:], in_=t1[:], func=AF.Abs,
                                         bias=zerof_col[:, 0:1], scale=1.0)
                    Sg = spool.tile([128, 160], BF16, tag="Sg")
                    nc.scalar.activation(out=Sg[:], in_=ta[:], func=AF.Relu,
                                         bias=onef_col[:, 0:1], scale=-1.0)
                    trb = pstr.tile([128, 128], BF16, tag="trb", space="PSUM")
                    nc.tensor.transpose(trb[:], Sg[:, 0:128], identb[:])
                    sta = med.tile([128, 128], BF16, tag="STa")
                    nc.vector.tensor_copy(out=sta[:], in_=trb[:])
                    trb2 = pstr.tile([128, 128], BF16, tag="trb", space="PSUM")
                    nc.tensor.transpose(trb2[0:32, :], Sg[:, 128:160], identb[:])
                    stb = med.tile([128, 128], BF16, tag="STb")
                    nc.vector.tensor_copy(out=stb[0:32, :], in_=trb2[0:32, :])
                    ps = psmm.tile([128, 512], F32, tag="mm", space="PSUM")
                    for j in range(SLG):
                        nc.tensor.matmul(ps[:, 128 * j:128 * (j + 1)],
                                         lhsT=x_lc[:, s, GC * g + 128 * j:GC * g + 128 * (j + 1)],
                                         rhs=sta[:], start=True, stop=False)
                        nc.tensor.matmul(ps[:, 128 * j:128 * (j + 1)],
                                         lhsT=x_lc[0:32, s + 1, GC * g + 128 * j:GC * g + 128 * (j + 1)],
                                         rhs=stb[0:32, :], start=False, stop=True)
                    nc.vector.tensor_copy(out=xs[:, SLG * g:SLG * (g + 1), 128 * s:128 * (s + 1)],
                                          in_=ps[:, 0:256])
        for g in range(G if not (NO_GATHER or MMG) else 0):
            tanh_t = sm.tile([128, NS], F32, tag="tanh")
            nc.scalar.activation(out=tanh_t[:], in_=offp[:, g * NS:(g + 1) * NS],
                                 func=AF.Tanh, bias=bcomp_sb[:, 0:1], scale=1.0)
            psm = sm.tile([128, NS], F32, tag="psm")
            nc.vector.tensor_scalar(out=psm[:], in0=tanh_t[:], scalar1=5.0 * c.sn,
                                    scalar2=None, op0=OP.mult)
            nc.vector.tensor_tensor(out=psm[:], in0=psm[:], in1=posg_sb[:], op=OP.add)
            if DBG and b == 0:
                nc.sync.dma_start(out=t['d_psm'].ap()[g], in_=psm[:])
            i0i = sm.tile([128, NS], I32, tag="i0i")
            nc.vector.tensor_copy(out=i0i[:], in_=psm[:])
            i0f = sm.tile([128, NS], F32, tag="i0f")
            nc.vector.tensor_copy(out=i0f[:], in_=i0i[:])
            w1 = sm.tile([128, NS], F32, tag="w1")
            nc.vector.tensor_tensor(out=w1[:], in0=psm[:], in1=i0f[:], op=OP.subtract)
            nc.vector.tensor_scalar(out=w1[:], in0=w1[:], scalar1=0.5, scalar2=None, op0=OP.add)
            adj = sm.tile([128, NS], F32, tag="adj")
            nc.vector.tensor_scalar(out=adj[:], in0=w1[:], scalar1=1.0, scalar2=None, op0=OP.is_ge)
            nc.vector.tensor_tensor(out=w1[:], in0=w1[:], in1=adj[:], op=OP.subtract)
            nc.vector.tensor_tensor(out=i0f[:], in0=i0f[:], in1=adj[:], op=OP.add)
            if DBG and b == 0:
                nc.sync.dma_start(out=t['d_w1'].ap()[g], in_=w1[:])
            idxf = sm.tile([128, NS], F32, tag="idxf")
            idx16 = sm.tile([128, 2 * NS], I16, tag="idx16")
            nc.vector.tensor_scalar(out=idxf[:], in0=i0f[:], scalar1=-10.0, scalar2=None, op0=OP.add)
            nc.vector.tensor_copy(out=idx16[:, 0:NS], in_=idxf[:])
            nc.vector.tensor_scalar(out=idxf[:], in0=i0f[:], scalar1=-9.0, scalar2=None, op0=OP.add)
            nc.vector.tensor_copy(out=idx16[:, NS:2 * NS], in_=idxf[:])

            # wrap idx to [16, NIDX/16] layout (replicated over partition groups) via DRAM
            NIDX = 2 * L
            scr = dscr.tile([128, 2 * NS], I16, tag="iscr")
            nc.sync.dma_start(out=scr[:], in_=idx16[:])
            wrapped = sm.tile([128, 16 * NS], I16, tag="wrap")
            scr_ap = scr[:]
            for kk in range(8):
                nc.sync.dma_start(out=wrapped[16 * kk:16 * (kk + 1), :], in_=bass.AP(
                    tensor=scr_ap.tensor, offset=scr_ap.offset,
                    ap=[[2 * NS, 16], [1, 2 * NS], [16 * 2 * NS, 8]]))

            # gather rows (i0 block | i1 block)
            g01 = med.tile([128, 2 * NS, GC], BF16, tag="g01")
            CH = min(1024, NIDX)
            for j in range(NIDX // CH):
                nc.gpsimd.dma_gather(
                    g01[:, (CH // 128) * j:(CH // 128) * (j + 1), :],
                    xg[b * LG:(b + 1) * LG, GC * g:GC * (g + 1)],
                    wrapped[:, (CH // 16) * j:(CH // 16) * (j + 1)],
                    CH, CH, GC, elem_step=C)

            # blend in place: g1 <- (g1 - g0) * w1 ; g0 <- g1 + g0
            nc.vector.tensor_tensor(out=g01[:, NS:2 * NS, :], in0=g01[:, NS:2 * NS, :],
                                    in1=g01[:, 0:NS, :], op=OP.subtract)
            for s in range(NS):
                nc.vector.tensor_scalar(out=g01[:, NS + s, :], in0=g01[:, NS + s, :],
                                        scalar1=w1[:, s:s + 1], scalar2=None, op0=OP.mult)
            nc.vector.tensor_tensor(out=g01[:, 0:NS, :], in0=g01[:, NS:2 * NS, :],
                                    in1=g01[:, 0:NS, :], op=OP.add)
            # transpose blended [l, c] blocks into xs [c, l]
            for s in range(NS):
                for j in range(SLG):
                    trp = pstr.tile([128, 128], F32, tag="tr", space="PSUM")
                    nc.tensor.transpose(trp[:], g01[:, s, 128 * j:128 * (j + 1)], identb[:])
                    nc.vector.tensor_copy(out=xs[:, SLG * g + j, 128 * s:128 * (s + 1)],
                                          in_=trp[:])

        if DBG and b == 0:
            nc.sync.dma_start(out=t['d_xs'].ap(), in_=xs[:])

        # ============== phase C: kT-pass, scores+softmax ==============
        kTt = big.tile([128, NS, C], BF16, tag="bigKA")
        for hi in range(NO):
            wk_h = load_whalf('wkT', hi)
            for lt in range(NS):
                ps = psmm.tile([128, 512], F32, tag="mm", space="PSUM")
                for kt in range(KT):
                    nc.tensor.matmul(ps[:], lhsT=xs[:, kt, 128 * lt:128 * (lt + 1)],
                                     rhs=wk_h[:, kt, :],
                                     start=(kt == 0), stop=(kt == KT - 1))
                nc.vector.tensor_tensor(out=kTt[:, lt, 512 * hi:512 * (hi + 1)],
                                        in0=ps[:], in1=bkr_bc[:, 512 * hi:512 * (hi + 1)],
                                        op=OP.add)

        if DBG and b == 0:
            nc.sync.dma_start(out=t['d_kT'].ap(), in_=kTt[:])
        # scores + softmax + transposed block-diag attn (pairs of heads)
        attnTs = []
        if NO_ATTN:
            for pr in range(c.P2):
                aT = sm.tile([128, 128], BF16, tag=f"aT{pr}")
                nc.vector.memset(aT[:], 0.0)
                attnTs.append(aT)
        for pr in range(c.P2 if not NO_ATTN else 0):
            # one [128x128] MM per lt covers both heads' diag blocks
            ps_sc = pssc.tile([128, 128], F32, tag="sc", space="PSUM")
            qsl = qsls[pr]
            for lt in range(NS):
                nc.tensor.matmul(ps_sc[:], lhsT=qsl[:, lt, :],
                                 rhs=kTt[:, lt, 128 * pr:128 * (pr + 1)],
                                 start=(lt == 0), stop=(lt == NS - 1))
            rmax = sm.tile([128, 1], F32, tag="rmax")
            nc.vector.reduce_max(out=rmax[0:64, :], in_=ps_sc[0:64, 0:64],
                                 axis=mybir.AxisListType.X)
            nc.vector.reduce_max(out=rmax[64:128, :], in_=ps_sc[64:128, 64:128],
                                 axis=mybir.AxisListType.X)
            nb_ = sm.tile([128, 1], F32, tag="nb")
            nc.vector.tensor_scalar(out=nb_[:], in0=rmax[:], scalar1=-scale, scalar2=None, op0=OP.mult)
            expt = sm.tile([128, 64], F32, tag="expt")
            nc.scalar.activation(out=expt[0:64, :], in_=ps_sc[0:64, 0:64], func=AF.Exp,
                                 bias=nb_[0:64, :], scale=scale)
            nc.scalar.activation(out=expt[64:128, :], in_=ps_sc[64:128, 64:128], func=AF.Exp,
                                 bias=nb_[64:128, :], scale=scale)
            rsum = sm.tile([128, 1], F32, tag="rsum")
            nc.vector.reduce_sum(out=rsum[:], in_=expt[:], axis=mybir.AxisListType.X)
            rinv = sm.tile([128, 1], F32, tag="rinv")
            nc.vector.reciprocal(out=rinv[:], in_=rsum[:])
            ablk = sm.tile([128, 128], F32, tag="ablk")
            nc.gpsimd.memset(ablk[:], 0.0)
            nc.vector.tensor_scalar(out=ablk[0:64, 0:64], in0=expt[0:64, :],
                                    scalar1=rinv[0:64, :], scalar2=None, op0=OP.mult)
            nc.vector.tensor_scalar(out=ablk[64:128, 64:128], in0=expt[64:128, :],
                                    scalar1=rinv[64:128, :], scalar2=None, op0=OP.mult)
            if DBG and b == 0:
                nc.sync.dma_start(out=t['d_ablk'].ap()[pr], in_=ablk[:])
            trp = pstr.tile([128, 128], F32, tag="tr", space="PSUM")
            nc.tensor.transpose(trp[:], ablk[:], ident[:])
            aT = sm.tile([128, 128], BF16, tag=f"aT{pr}")
            nc.vector.tensor_copy(out=aT[:], in_=trp[:])
            attnTs.append(aT)

        # ============== phase D: v-pass + attn@v ==============
        v = big.tile([128, KT, L], BF16, tag="bigGV")
        for hi in range(NO):
            wv_h = load_whalf('wvT', hi)
            for mm_ in range(MH):
                m = hi * MH + mm_
                for n in range(NB):
                    ps = psmm.tile([128, 512], F32, tag="mm", space="PSUM")
                    for kt in range(KT):
                        nc.tensor.matmul(ps[:], lhsT=wv_h[:, kt, 128 * mm_:128 * (mm_ + 1)],
                                         rhs=xs[:, kt, 512 * n:512 * (n + 1)],
                                         start=(kt == 0), stop=(kt == KT - 1))
                    rt = rp.tile([128, 512], BF16, tag="rt")
                    nc.sync.dma_start(out=rt[:], in_=t['rtab'].ap()[128 * m:128 * (m + 1),
                                                                    512 * n:512 * (n + 1)])
                    nc.vector.tensor_tensor(out=v[:, m, 512 * n:512 * (n + 1)],
                                            in0=ps[:], in1=rt[:], op=OP.add)

        if DBG and b == 0:
            nc.sync.dma_start(out=t['d_v'].ap(), in_=v[:])
        # attn @ v -> ao^T  [o, l]
        ao = big.tile([128, KT, L], BF16, tag="bigKA")
        for pr in range(c.P2):
            for n in range(NB):
                ps = psmm.tile([128, 512], F32, tag="mm", space="PSUM")
                nc.tensor.matmul(ps[:], lhsT=attnTs[pr][:],
                                 rhs=v[:, pr, 512 * n:512 * (n + 1)],
                                 start=True, stop=True)
                nc.vector.tensor_copy(out=ao[:, pr, 512 * n:512 * (n + 1)], in_=ps[:])

        if DBG and b == 0:
            nc.sync.dma_start(out=t['d_ao'].ap(), in_=ao[:])
        # ====== phase E: x-stationary out projection -> out2d [l, o] contiguous ======
        out_ap = t['out2d'].ap()
        for hi in range(NO):
            wo_h = load_whalf('woutT', hi)
            for lt in range(NS):
                ps = psmm.tile([128, 512], F32, tag="mm", space="PSUM")
                for kt in range(KT):
                    nc.tensor.matmul(ps[:], lhsT=ao[:, kt, 128 * lt:128 * (lt + 1)],
                                     rhs=wo_h[:, kt, :],
                                     start=(kt == 0), stop=(kt == KT - 1))
                yt = yp.tile([128, 512], F32, tag="yt")
                nc.vector.tensor_tensor(out=yt[:], in0=ps[:],
                                        in1=bor_bc[:, 512 * hi:512 * (hi + 1)],
                                        op=OP.add)
                nc.sync.dma_start(out=bass.AP(
                    tensor=out_ap.tensor, offset=(b * L + 128 * lt) * C + 512 * hi,
                    ap=[[C, 128], [1, 512]]), in_=yt[:])


def make_nc(cfg):
    nc = bacc.Bacc("TRN2", target_bir_lowering=False, debug=False)
    t = declare(nc, cfg)
    from contextlib import ExitStack
    with tile.TileContext(nc) as tc:
        with ExitStack() as ctx:
            build(tc, t, cfg, ctx)
    nc.compile()
    return nc


def host_prep_shared(inputs, cfg):
    c = cfg
    Wq, Wk, Wv, Wout = inputs['Wq'], inputs['Wk'], inputs['Wv'], inputs['Wout']
    w2 = np.asarray(inputs['Woff2'][0, :, 0], np.float32)          # [GC]
    W1 = np.asarray(inputs['Woff1'], np.float32)                   # [GC, GC, K]
    wpr = np.einsum('c,cik->ik', w2, W1)                           # [GC, K]
    w2b1 = float(np.dot(w2, np.asarray(inputs['boff1'], np.float32)))
    bcomp = w2b1 + float(np.asarray(inputs['boff2']).reshape(-1)[0])
    sh = {
        'wqT': np.ascontiguousarray(Wq.T).astype(BF),
        'wkT': np.ascontiguousarray(Wk.T).astype(BF),
        'wvT': np.ascontiguousarray(Wv.T).astype(BF),
        'woutT': np.ascontiguousarray(Wout.T).astype(BF),
        'wpr': np.ascontiguousarray(wpr).astype(BF),
        'bq_col': inputs['bq'][:, None].astype(np.float32),
        'bk_row': inputs['bk'][None, :].astype(BF),
        'bo_row': inputs['bout'][None, :].astype(BF),
        'bcompc': np.array([[bcomp]], np.float32),
        'w2b1c': np.array([[w2b1]], np.float32),
        'rtab': (inputs['bv'][:, None] + inputs['rpb_table'][0]).astype(BF),
        'posg': ((np.arange(128)[:, None] + 128 * np.arange(c.NS)[None, :]) * c.sn
                 + 15.0).astype(np.float32),
    }
    if getattr(c, 'mm_gather', False):
        sh['iotaR'] = np.broadcast_to(np.arange(160, dtype=np.float32)[None, :],
                                      (128, 160)).copy()
        sh['sub2'] = np.broadcast_to((6.5 + 128.0 * np.arange(c.NS, dtype=np.float32))[None, :],
                                     (128, c.NS)).copy()
    return sh


def host_prep_core(x_shard, cfg):
    c = cfg
    xgp = np.zeros((c.B_SH, c.L + c.GR, c.C), np.float32)
    xgp[:, c.GO:c.GO + c.L] = x_shard
    xt = np.swapaxes(np.asarray(x_shard, np.float32), 1, 2)        # [B_SH, C, L]
    return {
        'xgbf': xgp.reshape(c.B_SH * (c.L + c.GR), c.C).astype(BF),
        'xtbf': np.ascontiguousarray(xt).reshape(c.B_SH * c.C, c.L).astype(BF),
    }


# ----------------------------------------------------------------------------
# Public entry point
# ----------------------------------------------------------------------------
_N_CORES = 8
_B, _L, _C, _H, _G, _K = 16, 2048, 1024, 16, 4, 5
_CACHE = {}


def _get_nc(cfg):
    if 'nc' not in _CACHE:
        _CACHE['nc'] = make_nc(cfg)
    return _CACHE['nc']


def kernel(**inputs):
    inputs = {k: np.asarray(v) for k, v in inputs.items()}
    cfg = Cfg(B_SH=_B // _N_CORES, L=_L, C=_C, H=_H, G=_G, K=_K)
    cfg.set_mm_gather(False)
    nc = _get_nc(cfg)
    sh = host_prep_shared(inputs, cfg)
    in_maps = [
        {**sh, **host_prep_core(inputs['x'][c * cfg.B_SH:(c + 1) * cfg.B_SH], cfg)}
        for c in range(_N_CORES)
    ]
    from concourse.bass_utils import run_bass_kernel_spmd
    res = run_bass_kernel_spmd(nc, in_maps, core_ids=list(range(_N_CORES)))
    out = np.concatenate(
        [res.results[c]["out2d"].reshape(cfg.B_SH, _L, _C) for c in range(_N_CORES)],
        axis=0)
    return out.astype(np.float32)


# revision 4
# speedup vs baseline: 3.3519x; 3.3519x over previous
"""v2 Trainium2 Bass kernel for nn_DeformAtten1D.

Changes vs baseline:
- xT fed pre-transposed from host (contiguous loads, no rearranged-AP DMA)
- qT via on-chip tensor transposes of gq (drops the duplicate q projection
  and the qt_scr DRAM round trip)
- offset network collapsed to a single 1-channel conv (W' = w2^T @ W1),
  with boundary correction for the conv2 zero-padding
- x_sampled kept in SBUF via on-chip transposes (drops xs_scr round trip)
- output projection x-stationary so stores are contiguous [l, o]
"""
import sys
for _p in ('/opt/trn_rl_repo', '/root/.axon_site/_ro/trn_rl_repo'):
    if _p not in sys.path:
        sys.path.insert(0, _p)

import numpy as np
import ml_dtypes

import concourse.bass as bass
import concourse.bacc as bacc
import concourse.mybir as mybir
import concourse.tile as tile
from concourse.masks import make_identity
from concourse.library_config import mlp

F32 = mybir.dt.float32
BF16 = mybir.dt.bfloat16
I32 = mybir.dt.int32
I16 = mybir.dt.int16
AF = mybir.ActivationFunctionType
OP = mybir.AluOpType
BF = ml_dtypes.bfloat16


class Cfg:
    def set_mm_gather(self, on=True):
        self.mm_gather = on
        if on:
            self.GO, self.GR = 9, 32
        else:
            self.GO, self.GR = 6, 12

    def __init__(self, B_SH, L, C, H, G, K=5):
        self.B_SH, self.L, self.C, self.H, self.G, self.K = B_SH, L, C, H, G, K
        self.GC = C // G
        self.DH = C // H
        assert self.DH == 64
        self.PAD = K // 2
        self.L4 = L + 2 * self.PAD
        self.sn = L / (self.L4 - 1)
        self.KT = C // 128
        self.NS = L // 128
        self.SLG = self.GC // 128
        self.NB = L // 512
        self.NO = C // 512
        self.MH = self.KT // self.NO   # m-tiles per weight half-block
        self.GR = 12
        self.GO = 6
        if getattr(self, 'mm_gather', False):
            self.GO, self.GR = 9, 32
        self.P2 = H // 2
        assert self.P2 == self.KT
        assert L % 512 == 0 and C % 512 == 0 and self.GC % 128 == 0


def declare(nc, cfg):
    c = cfg
    t = {}
    t['xgbf'] = nc.dram_tensor("xgbf", [c.B_SH * (c.L + c.GR), c.C], BF16, kind="ExternalInput")
    t['xtbf'] = nc.dram_tensor("xtbf", [c.B_SH * c.C, c.L], BF16, kind="ExternalInput")
    t['rtab'] = nc.dram_tensor("rtab", [c.C, c.L], BF16, kind="ExternalInput")
    t['wpr'] = nc.dram_tensor("wpr", [c.GC, c.K], BF16, kind="ExternalInput")
    for nm in ('wqT', 'wkT', 'wvT', 'woutT'):
        t[nm] = nc.dram_tensor(nm, [c.C, c.C], BF16, kind="ExternalInput")
    t['bq_col'] = nc.dram_tensor("bq_col", [c.C, 1], F32, kind="ExternalInput")
    t['bk_row'] = nc.dram_tensor("bk_row", [1, c.C], BF16, kind="ExternalInput")
    t['bo_row'] = nc.dram_tensor("bo_row", [1, c.C], BF16, kind="ExternalInput")
    t['bcompc'] = nc.dram_tensor("bcompc", [1, 1], F32, kind="ExternalInput")
    t['w2b1c'] = nc.dram_tensor("w2b1c", [1, 1], F32, kind="ExternalInput")
    t['posg'] = nc.dram_tensor("posg", [128, c.NS], F32, kind="ExternalInput")
    if getattr(c, 'mm_gather', False):
        t['iotaR'] = nc.dram_tensor("iotaR", [128, 160], F32, kind="ExternalInput")
        t['sub2'] = nc.dram_tensor("sub2", [128, c.NS], F32, kind="ExternalInput")
    t['out2d'] = nc.dram_tensor("out2d", [c.B_SH * c.L, c.C], BF16, kind="ExternalOutput")
    dbg = getattr(c, 'debug', False)
    if dbg:
        t['d_gq'] = nc.dram_tensor("d_gq", [128, c.KT, c.L + 4], BF16, kind="ExternalOutput")
        t['d_offrow'] = nc.dram_tensor("d_offrow", [c.G, c.L], F32, kind="ExternalOutput")
        t['d_psm'] = nc.dram_tensor("d_psm", [c.G, 128, c.NS], F32, kind="ExternalOutput")
        t['d_w1'] = nc.dram_tensor("d_w1", [c.G, 128, c.NS], F32, kind="ExternalOutput")
        t['d_xs'] = nc.dram_tensor("d_xs", [128, c.KT, c.L], BF16, kind="ExternalOutput")
        t['d_kT'] = nc.dram_tensor("d_kT", [128, c.NS, c.C], BF16, kind="ExternalOutput")
        t['d_qsl'] = nc.dram_tensor("d_qsl", [c.P2, 128, c.NS, 128], BF16, kind="ExternalOutput")
        t['d_ablk'] = nc.dram_tensor("d_ablk", [c.P2, 128, 128], F32, kind="ExternalOutput")
        t['d_v'] = nc.dram_tensor("d_v", [128, c.KT, c.L], BF16, kind="ExternalOutput")
        t['d_ao'] = nc.dram_tensor("d_ao", [128, c.KT, c.L], BF16, kind="ExternalOutput")
    return t


def build(tc, t, cfg, ctx):
    c = cfg
    nc = tc.nc
    L, C, KT, NS, NB, NO, GC, SLG, G, K, MH = (c.L, c.C, c.KT, c.NS, c.NB, c.NO,
                                               c.GC, c.SLG, c.G, c.K, c.MH)
    scale = C ** -0.5

    nc.gpsimd.load_library(mlp)

    konst = ctx.enter_context(tc.tile_pool(name="konst", bufs=1))
    big = ctx.enter_context(tc.tile_pool(name="big", bufs=1))
    wp = ctx.enter_context(tc.tile_pool(name="wp", bufs=2))
    med = ctx.enter_context(tc.tile_pool(name="med", bufs=2))
    sm = ctx.enter_context(tc.tile_pool(name="sm", bufs=2))
    rp = ctx.enter_context(tc.tile_pool(name="rp", bufs=2))
    yp = ctx.enter_context(tc.tile_pool(name="yp", bufs=3))
    qp = ctx.enter_context(tc.tile_pool(name="qp", bufs=1))
    orow = ctx.enter_context(tc.tile_pool(name="orow", bufs=3))
    spool = ctx.enter_context(tc.tile_pool(name="spool", bufs=3))
    dscr = ctx.enter_context(tc.tile_pool(name="dscr", bufs=2, space="DRAM"))
    psmm = ctx.enter_context(tc.tile_pool(name="psmm", bufs=3, space="PSUM"))
    pssc = ctx.enter_context(tc.tile_pool(name="pssc", bufs=2, space="PSUM"))
    pstr = ctx.enter_context(tc.tile_pool(name="pstr", bufs=2, space="PSUM"))
    psoff2 = ctx.enter_context(tc.tile_pool(name="psoff2", bufs=1, space="PSUM"))

    # ---- constants ----
    wpr_sb = konst.tile([128, SLG, K], BF16, tag="wpr")
    nc.sync.dma_start(out=wpr_sb[:], in_=bass.AP(
        tensor=t['wpr'].ap().tensor, offset=0,
        ap=[[K, 128], [128 * K, SLG], [1, K]]))
    posg_sb = konst.tile([128, NS], F32, tag="posg")
    nc.sync.dma_start(out=posg_sb[:], in_=t['posg'].ap())
    bq_col_sb = konst.tile([128, KT], F32, tag="bqc")
    nc.sync.dma_start(out=bq_col_sb[:], in_=bass.AP(
        tensor=t['bq_col'].ap().tensor, offset=0, ap=[[1, 128], [128, KT]]))
    bcomp_sb = konst.tile([128, 1], F32, tag="bco")
    nc.sync.dma_start(out=bcomp_sb[:], in_=bass.AP(
        tensor=t['bcompc'].ap().tensor, offset=0, ap=[[0, 128], [1, 1]]))
    w2b1_sb = konst.tile([G, 1], F32, tag="w2b1")
    nc.sync.dma_start(out=w2b1_sb[:], in_=bass.AP(
        tensor=t['w2b1c'].ap().tensor, offset=0, ap=[[0, G], [1, 1]]))
    bkr_sb = konst.tile([1, C], BF16, tag="bkr")
    nc.sync.dma_start(out=bkr_sb[:], in_=t['bk_row'].ap())
    bor_sb = konst.tile([1, C], BF16, tag="bor")
    nc.sync.dma_start(out=bor_sb[:], in_=t['bo_row'].ap())
    bkr_bc = konst.tile([128, C], BF16, tag="bkrb")
    nc.gpsimd.partition_broadcast(bkr_bc[:], bkr_sb[:])
    bor_bc = konst.tile([128, C], BF16, tag="borb")
    nc.gpsimd.partition_broadcast(bor_bc[:], bor_sb[:])
    ones_sb = konst.tile([1, 128], BF16, tag="ones")
    nc.vector.memset(ones_sb[:], 1.0)
    ones_col = konst.tile([G, 1], F32, tag="onesc")
    nc.vector.memset(ones_col[:], 1.0)
    onef_col = konst.tile([128, 1], F32, tag="onef")
    nc.vector.memset(onef_col[:], 1.0)
    zerof_col = konst.tile([128, 1], F32, tag="zerof")
    nc.vector.memset(zerof_col[:], 0.0)
    MMG = getattr(c, 'mm_gather', False)
    if MMG:
        iotaR_sb = konst.tile([128, 160], F32, tag="iotaR")
        nc.sync.dma_start(out=iotaR_sb[:], in_=t['iotaR'].ap())
        sub2_sb = konst.tile([128, NS], F32, tag="sub2")
        nc.sync.dma_start(out=sub2_sb[:], in_=t['sub2'].ap())
    ident = konst.tile([128, 128], F32, tag="ident")
    make_identity(nc, ident[:])
    identb = konst.tile([128, 128], BF16, tag="identb")
    make_identity(nc, identb[:])

    DBG = getattr(c, 'debug', False)
    NO_GATHER = getattr(c, 'no_gather', False)
    NO_CONV = getattr(c, 'no_conv', False)
    NO_QT = getattr(c, 'no_qt', False)
    NO_ATTN = getattr(c, 'no_attn', False)
    xg = t['xgbf'].ap()
    LG = L + c.GR

    def load_whalf(wname, hi):
        wh = wp.tile([128, KT, 512], BF16, tag="wblk")
        nc.sync.dma_start(out=wh[:], in_=bass.AP(
            tensor=t[wname].ap().tensor, offset=512 * hi,
            ap=[[C, 128], [128 * C, KT], [1, 512]]))
        return wh

    REPEAT = getattr(c, 'repeat', 1)
    for b in [bb for _ in range(REPEAT) for bb in range(c.B_SH)]:
        # ================= phase A: xT load (contiguous), q-pass =================
        xT = big.tile([128, KT, L], BF16, tag="bigX")
        for kt in range(KT):
            nc.sync.dma_start(
                out=xT[:, kt, :],
                in_=t['xtbf'].ap()[b * C + 128 * kt: b * C + 128 * (kt + 1), :])

        gq = big.tile([128, KT, L + 4], BF16, tag="bigGV")
        nc.gpsimd.memset(gq[:, :, 0:4], 0.0)
        for hi in range(NO):
            wq_h = load_whalf('wqT', hi)
            for mm_ in range(MH):
                m = hi * MH + mm_
                for n in range(NB):
                    ps = psmm.tile([128, 512], F32, tag="mm", space="PSUM")
                    for kt in range(KT):
                        nc.tensor.matmul(ps[:], lhsT=wq_h[:, kt, 128 * mm_:128 * (mm_ + 1)],
                                         rhs=xT[:, kt, 512 * n:512 * (n + 1)],
                                         start=(kt == 0), stop=(kt == KT - 1))
                    nc.scalar.activation(out=gq[:, m, 4 + 512 * n:4 + 512 * (n + 1)], in_=ps[:],
                                         func=AF.Identity, bias=bq_col_sb[:, m:m + 1], scale=1.0)

        if DBG and b == 0:
            nc.sync.dma_start(out=t['d_gq'].ap(), in_=gq[:])

        # ====== phase B0: composite offset conv -> off rows [G, L] ======
        # off[g, l'] = sum_{cin,t} W'[cin,t] * gq_pad[g*GC+cin, l'-4+t]  (+bias via tanh)
        # single-partition chunks (matmul base-partition rule), consumed
        # immediately into offp so only a small rotating row buffer is needed
        offp = psoff2.tile([128, G * NS], F32, tag="offp", space="PSUM")
        for g in range(G if not NO_CONV else 0):
            for n in range(NB):
                po = psmm.tile([128, 512], F32, tag="mm", space="PSUM")
                first = True
                for kt2 in range(SLG):
                    for tt_ in range(K):
                        nc.tensor.matmul(
                            po[0:1, :],
                            lhsT=wpr_sb[:, kt2, tt_:tt_ + 1],
                            rhs=gq[:, g * SLG + kt2, 512 * n + tt_: 512 * n + tt_ + 512],
                            start=first, stop=(kt2 == SLG - 1 and tt_ == K - 1))
                        first = False
                oc = orow.tile([1, 512], F32, tag="oc")
                nc.vector.tensor_copy(out=oc[0:1, :], in_=po[0:1, :])
                if n == 0:
                    # conv2 zero-pad boundary: at l' in {0,1} the inner conv
                    # reads zero-padding, so off = b2 there: overwrite the
                    # composite sum with -w2b1 (tanh bias adds bcomp back)
                    nc.vector.tensor_scalar(out=oc[0:1, 0:2], in0=oc[0:1, 0:2],
                                            scalar1=0.0, scalar2=None, op0=OP.mult)
                    nc.vector.tensor_scalar(out=oc[0:1, 0:2], in0=oc[0:1, 0:2],
                                            scalar1=w2b1_sb[0:1, :], scalar2=None,
                                            op0=OP.subtract)
                if DBG and b == 0:
                    nc.sync.dma_start(out=bass.AP(
                        tensor=t['d_offrow'].ap().tensor, offset=g * L + 512 * n,
                        ap=[[512, 1], [1, 512]]), in_=oc[0:1, :])
                # transpose chunk -> offp cols (col g*NS+s holds l = 128s+p)
                for k4 in range(4):
                    s = 4 * n + k4
                    nc.tensor.matmul(offp[:, g * NS + s:g * NS + s + 1],
                                     lhsT=oc[0:1, 128 * k4:128 * (k4 + 1)],
                                     rhs=ones_col[0:1, :],
                                     start=True, stop=True)

        if NO_CONV:
            nc.vector.memset(offp[:], 0.0)
        # ====== phase B1: qT transposes (independent of gather; fills tensor) ======
        qsls = []
        for pr in range(c.P2):
            qsl = qp.tile([128, NS, 128], BF16, tag=f"qsl{pr}")
            for lt in range(NS if not NO_QT else 0):
                trp = pstr.tile([128, 128], F32, tag="tr", space="PSUM")
                nc.tensor.transpose(trp[:], gq[:, pr, 4 + 128 * lt:4 + 128 * (lt + 1)], identb[:])
                nc.vector.tensor_copy(out=qsl[:, lt, :], in_=trp[:])
            qsls.append(qsl)
            if DBG and b == 0:
                nc.sync.dma_start(out=t['d_qsl'].ap()[pr], in_=qsl[:])

        # ====== phase B2: per group offsets -> sample -> xs ======
        if NO_GATHER:
            xs = xT
        else:
            xs = big.tile([128, KT, L], BF16, tag="bigX")
        if MMG and not NO_GATHER:
            # banded-matmul bilinear sampler: xs[c,l'] built directly via
            # psum = x_window[r,c].T @ S^T[r,l'], S[l,r] = -relu(1-|r-posrel|)
            x_lc = big.tile([128, NS + 1, C], BF16, tag="bigKA")
            for s in range(NS):
                nc.sync.dma_start(out=x_lc[:, s, :],
                                  in_=xg[b * LG + 128 * s: b * LG + 128 * (s + 1), :])
            nc.sync.dma_start(out=x_lc[0:32, NS, :],
                              in_=xg[b * LG + 128 * NS: b * LG + 128 * NS + 32, :])
            for g in range(G):
                tanh_t = sm.tile([128, NS], F32, tag="tanh")
                nc.scalar.activation(out=tanh_t[:], in_=offp[:, g * NS:(g + 1) * NS],
                                     func=AF.Tanh, bias=bcomp_sb[:, 0:1], scale=1.0)
                psm = sm.tile([128, NS], F32, tag="psm")
                nc.vector.tensor_scalar(out=psm[:], in0=tanh_t[:], scalar1=5.0 * c.sn,
                                        scalar2=None, op0=OP.mult)
                nc.vector.tensor_tensor(out=psm[:], in0=psm[:], in1=posg_sb[:], op=OP.add)
                if DBG and b == 0:
                    nc.sync.dma_start(out=t['d_psm'].ap()[g], in_=psm[:])
                prel = sm.tile([128, NS], F32, tag="prel")
                nc.vector.tensor_tensor(out=prel[:], in0=psm[:], in1=sub2_sb[:],
                                        op=OP.subtract)
                for s in range(NS):
                    # S = relu(1 - |d|), d = r - posrel  (hat weights, via ACT)
                    t1 = spool.tile([128, 160], F32, tag="t1")
                    nc.vector.tensor_scalar(out=t1[:], in0=iotaR_sb[:],
                                            scalar1=prel[:, s:s + 1], scalar2=None,
                                            op0=OP.subtract)
                    ta = spool.tile([128, 160], F32, tag="ta")
                    nc.scalar.activation(out=ta[:], in_=t1[:], func=AF.Abs,
                                         bias=zerof_col[:, 0:1], scale=1.0)
                    Sg = spool.tile([128, 160], BF16, tag="Sg")
                    nc.scalar.activation(out=Sg[:], in_=ta[:], func=AF.Relu,
                                         bias=onef_col[:, 0:1], scale=-1.0)
                    trb = pstr.tile([128, 128], BF16, tag="trb", space="PSUM")
                    nc.tensor.transpose(trb[:], Sg[:, 0:128], identb[:])
                    sta = med.tile([128, 128], BF16, tag="STa")
                    nc.vector.tensor_copy(out=sta[:], in_=trb[:])
                    trb2 = pstr.tile([128, 128], BF16, tag="trb", space="PSUM")
                    nc.tensor.transpose(trb2[0:32, :], Sg[:, 128:160], identb[:])
                    stb = med.tile([128, 128], BF16, tag="STb")
                    nc.vector.tensor_copy(out=stb[0:32, :], in_=trb2[0:32, :])
                    ps = psmm.tile([128, 512], F32, tag="mm", space="PSUM")
                    for j in range(SLG):
                        nc.tensor.matmul(ps[:, 128 * j:128 * (j + 1)],
                                         lhsT=x_lc[:, s, GC * g + 128 * j:GC * g + 128 * (j + 1)],
                                         rhs=sta[:], start=True, stop=False)
                        nc.tensor.matmul(ps[:, 128 * j:128 * (j + 1)],
                                         lhsT=x_lc[0:32, s + 1, GC * g + 128 * j:GC * g + 128 * (j + 1)],
                                         rhs=stb[0:32, :], start=False, stop=True)
                    nc.vector.tensor_copy(out=xs[:, SLG * g:SLG * (g + 1), 128 * s:128 * (s + 1)],
                                          in_=ps[:, 0:256])
        for g in range(G if not (NO_GATHER or MMG) else 0):
            tanh_t = sm.tile([128, NS], F32, tag="tanh")
            nc.scalar.activation(out=tanh_t[:], in_=offp[:, g * NS:(g + 1) * NS],
                                 func=AF.Tanh, bias=bcomp_sb[:, 0:1], scale=1.0)
            psm = sm.tile([128, NS], F32, tag="psm")
            nc.vector.tensor_scalar(out=psm[:], in0=tanh_t[:], scalar1=5.0 * c.sn,
                                    scalar2=None, op0=OP.mult)
            nc.vector.tensor_tensor(out=psm[:], in0=psm[:], in1=posg_sb[:], op=OP.add)
            if DBG and b == 0:
                nc.sync.dma_start(out=t['d_psm'].ap()[g], in_=psm[:])
            i0i = sm.tile([128, NS], I32, tag="i0i")
            nc.vector.tensor_copy(out=i0i[:], in_=psm[:])
            i0f = sm.tile([128, NS], F32, tag="i0f")
            nc.vector.tensor_copy(out=i0f[:], in_=i0i[:])
            w1 = sm.tile([128, NS], F32, tag="w1")
            nc.vector.tensor_tensor(out=w1[:], in0=psm[:], in1=i0f[:], op=OP.subtract)
            nc.vector.tensor_scalar(out=w1[:], in0=w1[:], scalar1=0.5, scalar2=None, op0=OP.add)
            adj = sm.tile([128, NS], F32, tag="adj")
            nc.vector.tensor_scalar(out=adj[:], in0=w1[:], scalar1=1.0, scalar2=None, op0=OP.is_ge)
            nc.vector.tensor_tensor(out=w1[:], in0=w1[:], in1=adj[:], op=OP.subtract)
            nc.vector.tensor_tensor(out=i0f[:], in0=i0f[:], in1=adj[:], op=OP.add)
            if DBG and b == 0:
                nc.sync.dma_start(out=t['d_w1'].ap()[g], in_=w1[:])
            idxf = sm.tile([128, NS], F32, tag="idxf")
            idx16 = sm.tile([128, 2 * NS], I16, tag="idx16")
            nc.vector.tensor_scalar(out=idxf[:], in0=i0f[:], scalar1=-10.0, scalar2=None, op0=OP.add)
            nc.vector.tensor_copy(out=idx16[:, 0:NS], in_=idxf[:])
            nc.vector.tensor_scalar(out=idxf[:], in0=i0f[:], scalar1=-9.0, scalar2=None, op0=OP.add)
            nc.vector.tensor_copy(out=idx16[:, NS:2 * NS], in_=idxf[:])

            # wrap idx to [16, NIDX/16] layout (replicated over partition groups) via DRAM
            NIDX = 2 * L
            scr = dscr.tile([128, 2 * NS], I16, tag="iscr")
            nc.sync.dma_start(out=scr[:], in_=idx16[:])
            wrapped = sm.tile([128, 16 * NS], I16, tag="wrap")
            scr_ap = scr[:]
            for kk in range(8):
                nc.sync.dma_start(out=wrapped[16 * kk:16 * (kk + 1), :], in_=bass.AP(
                    tensor=scr_ap.tensor, offset=scr_ap.offset,
                    ap=[[2 * NS, 16], [1, 2 * NS], [16 * 2 * NS, 8]]))

            # gather rows (i0 block | i1 block)
            g01 = med.tile([128, 2 * NS, GC], BF16, tag="g01")
            CH = min(1024, NIDX)
            for j in range(NIDX // CH):
                nc.gpsimd.dma_gather(
                    g01[:, (CH // 128) * j:(CH // 128) * (j + 1), :],
                    xg[b * LG:(b + 1) * LG, GC * g:GC * (g + 1)],
                    wrapped[:, (CH // 16) * j:(CH // 16) * (j + 1)],
                    CH, CH, GC, elem_step=C)

            # blend in place: g1 <- (g1 - g0) * w1 ; g0 <- g1 + g0
            nc.vector.tensor_tensor(out=g01[:, NS:2 * NS, :], in0=g01[:, NS:2 * NS, :],
                                    in1=g01[:, 0:NS, :], op=OP.subtract)
            for s in range(NS):
                nc.vector.tensor_scalar(out=g01[:, NS + s, :], in0=g01[:, NS + s, :],
                                        scalar1=w1[:, s:s + 1], scalar2=None, op0=OP.mult)
            nc.vector.tensor_tensor(out=g01[:, 0:NS, :], in0=g01[:, NS:2 * NS, :],
                                    in1=g01[:, 0:NS, :], op=OP.add)
            # transpose blended [l, c] blocks into xs [c, l]
            for s in range(NS):
                for j in range(SLG):
                    trp = pstr.tile([128, 128], F32, tag="tr", space="PSUM")
                    nc.tensor.transpose(trp[:], g01[:, s, 128 * j:128 * (j + 1)], identb[:])
                    nc.vector.tensor_copy(out=xs[:, SLG * g + j, 128 * s:128 * (s + 1)],
                                          in_=trp[:])

        if DBG and b == 0:
            nc.sync.dma_start(out=t['d_xs'].ap(), in_=xs[:])

        # ============== phase C: kT-pass, scores+softmax ==============
        kTt = big.tile([128, NS, C], BF16, tag="bigKA")
        for hi in range(NO):
            wk_h = load_whalf('wkT', hi)
            for lt in range(NS):
                ps = psmm.tile([128, 512], F32, tag="mm", space="PSUM")
                for kt in range(KT):
                    nc.tensor.matmul(ps[:], lhsT=xs[:, kt, 128 * lt:128 * (lt + 1)],
                                     rhs=wk_h[:, kt, :],
                                     start=(kt == 0), stop=(kt == KT - 1))
                nc.vector.tensor_tensor(out=kTt[:, lt, 512 * hi:512 * (hi + 1)],
                                        in0=ps[:], in1=bkr_bc[:, 512 * hi:512 * (hi + 1)],
                                        op=OP.add)

        if DBG and b == 0:
            nc.sync.dma_start(out=t['d_kT'].ap(), in_=kTt[:])
        # scores + softmax + transposed block-diag attn (pairs of heads)
        attnTs = []
        if NO_ATTN:
            for pr in range(c.P2):
                aT = sm.tile([128, 128], BF16, tag=f"aT{pr}")
                nc.vector.memset(aT[:], 0.0)
                attnTs.append(aT)
        for pr in range(c.P2 if not NO_ATTN else 0):
            # one [128x128] MM per lt covers both heads' diag blocks
            ps_sc = pssc.tile([128, 128], F32, tag="sc", space="PSUM")
            qsl = qsls[pr]
            for lt in range(NS):
                nc.tensor.matmul(ps_sc[:], lhsT=qsl[:, lt, :],
                                 rhs=kTt[:, lt, 128 * pr:128 * (pr + 1)],
                                 start=(lt == 0), stop=(lt == NS - 1))
            rmax = sm.tile([128, 1], F32, tag="rmax")
            nc.vector.reduce_max(out=rmax[0:64, :], in_=ps_sc[0:64, 0:64],
                                 axis=mybir.AxisListType.X)
            nc.vector.reduce_max(out=rmax[64:128, :], in_=ps_sc[64:128, 64:128],
                                 axis=mybir.AxisListType.X)
            nb_ = sm.tile([128, 1], F32, tag="nb")
            nc.vector.tensor_scalar(out=nb_[:], in0=rmax[:], scalar1=-scale, scalar2=None, op0=OP.mult)
            expt = sm.tile([128, 64], F32, tag="expt")
            nc.scalar.activation(out=expt[0:64, :], in_=ps_sc[0:64, 0:64], func=AF.Exp,
                                 bias=nb_[0:64, :], scale=scale)
            nc.scalar.activation(out=expt[64:128, :], in_=ps_sc[64:128, 64:128], func=AF.Exp,
                                 bias=nb_[64:128, :], scale=scale)
            rsum = sm.tile([128, 1], F32, tag="rsum")
            nc.vector.reduce_sum(out=rsum[:], in_=expt[:], axis=mybir.AxisListType.X)
            rinv = sm.tile([128, 1], F32, tag="rinv")
            nc.vector.reciprocal(out=rinv[:], in_=rsum[:])
            ablk = sm.tile([128, 128], F32, tag="ablk")
            nc.gpsimd.memset(ablk[:], 0.0)
            nc.vector.tensor_scalar(out=ablk[0:64, 0:64], in0=expt[0:64, :],
                                    scalar1=rinv[0:64, :], scalar2=None, op0=OP.mult)
            nc.vector.tensor_scalar(out=ablk[64:128, 64:128], in0=expt[64:128, :],
                                    scalar1=rinv[64:128, :], scalar2=None, op0=OP.mult)
            if DBG and b == 0:
                nc.sync.dma_start(out=t['d_ablk'].ap()[pr], in_=ablk[:])
            trp = pstr.tile([128, 128], F32, tag="tr", space="PSUM")
            nc.tensor.transpose(trp[:], ablk[:], ident[:])
            aT = sm.tile([128, 128], BF16, tag=f"aT{pr}")
            nc.vector.tensor_copy(out=aT[:], in_=trp[:])
            attnTs.append(aT)

        # ============== phase D: v-pass + attn@v ==============
        v = big.tile([128, KT, L], BF16, tag="bigGV")
        for hi in range(NO):
            wv_h = load_whalf('wvT', hi)
            for mm_ in range(MH):
                m = hi * MH + mm_
                for n in range(NB):
                    ps = psmm.tile([128, 512], F32, tag="mm", space="PSUM")
                    for kt in range(KT):
                        nc.tensor.matmul(ps[:], lhsT=wv_h[:, kt, 128 * mm_:128 * (mm_ + 1)],
                                         rhs=xs[:, kt, 512 * n:512 * (n + 1)],
                                         start=(kt == 0), stop=(kt == KT - 1))
                    rt = rp.tile([128, 512], BF16, tag="rt")
                    nc.sync.dma_start(out=rt[:], in_=t['rtab'].ap()[128 * m:128 * (m + 1),
                                                                    512 * n:512 * (n + 1)])
                    nc.vector.tensor_tensor(out=v[:, m, 512 * n:512 * (n + 1)],
                                            in0=ps[:], in1=rt[:], op=OP.add)

        if DBG and b == 0:
            nc.sync.dma_start(out=t['d_v'].ap(), in_=v[:])
        # attn @ v -> ao^T  [o, l]
        ao = big.tile([128, KT, L], BF16, tag="bigKA")
        for pr in range(c.P2):
            for n in range(NB):
                ps = psmm.tile([128, 512], F32, tag="mm", space="PSUM")
                nc.tensor.matmul(ps[:], lhsT=attnTs[pr][:],
                                 rhs=v[:, pr, 512 * n:512 * (n + 1)],
                                 start=True, stop=True)
                nc.vector.tensor_copy(out=ao[:, pr, 512 * n:512 * (n + 1)], in_=ps[:])

        if DBG and b == 0:
            nc.sync.dma_start(out=t['d_ao'].ap(), in_=ao[:])
        # ====== phase E: x-stationary out projection -> out2d [l, o] contiguous ======
        out_ap = t['out2d'].ap()
        for hi in range(NO):
            wo_h = load_whalf('woutT', hi)
            for lt in range(NS):
                ps = psmm.tile([128, 512], F32, tag="mm", space="PSUM")
                for kt in range(KT):
                    nc.tensor.matmul(ps[:], lhsT=ao[:, kt, 128 * lt:128 * (lt + 1)],
                                     rhs=wo_h[:, kt, :],
                                     start=(kt == 0), stop=(kt == KT - 1))
                yt = yp.tile([128, 512], BF16, tag="yt")
                nc.vector.tensor_tensor(out=yt[:], in0=ps[:],
                                        in1=bor_bc[:, 512 * hi:512 * (hi + 1)],
                                        op=OP.add)
                nc.sync.dma_start(out=bass.AP(
                    tensor=out_ap.tensor, offset=(b * L + 128 * lt) * C + 512 * hi,
                    ap=[[C, 128], [1, 512]]), in_=yt[:])


def make_nc(cfg):
    nc = bacc.Bacc("TRN2", target_bir_lowering=False, debug=False)
    t = declare(nc, cfg)
    from contextlib import ExitStack
    with tile.TileContext(nc) as tc:
        with ExitStack() as ctx:
            build(tc, t, cfg, ctx)
    nc.compile()
    return nc


def host_prep_shared(inputs, cfg):
    c = cfg
    Wq, Wk, Wv, Wout = inputs['Wq'], inputs['Wk'], inputs['Wv'], inputs['Wout']
    w2 = np.asarray(inputs['Woff2'][0, :, 0], np.float32)          # [GC]
    W1 = np.asarray(inputs['Woff1'], np.float32)                   # [GC, GC, K]
    wpr = np.einsum('c,cik->ik', w2, W1)                           # [GC, K]
    w2b1 = float(np.dot(w2, np.asarray(inputs['boff1'], np.float32)))
    bcomp = w2b1 + float(np.asarray(inputs['boff2']).reshape(-1)[0])
    sh = {
        'wqT': np.ascontiguousarray(Wq.T).astype(BF),
        'wkT': np.ascontiguousarray(Wk.T).astype(BF),
        'wvT': np.ascontiguousarray(Wv.T).astype(BF),
        'woutT': np.ascontiguousarray(Wout.T).astype(BF),
        'wpr': np.ascontiguousarray(wpr).astype(BF),
        'bq_col': inputs['bq'][:, None].astype(np.float32),
        'bk_row': inputs['bk'][None, :].astype(BF),
        'bo_row': inputs['bout'][None, :].astype(BF),
        'bcompc': np.array([[bcomp]], np.float32),
        'w2b1c': np.array([[w2b1]], np.float32),
        'rtab': (inputs['bv'][:, None] + inputs['rpb_table'][0]).astype(BF),
        'posg': ((np.arange(128)[:, None] + 128 * np.arange(c.NS)[None, :]) * c.sn
                 + 15.0).astype(np.float32),
    }
    if getattr(c, 'mm_gather', False):
        sh['iotaR'] = np.broadcast_to(np.arange(160, dtype=np.float32)[None, :],
                                      (128, 160)).copy()
        sh['sub2'] = np.broadcast_to((6.5 + 128.0 * np.arange(c.NS, dtype=np.float32))[None, :],
                                     (128, c.NS)).copy()
    return sh


def host_prep_core(x_shard, cfg):
    c = cfg
    xgp = np.zeros((c.B_SH, c.L + c.GR, c.C), np.float32)
    xgp[:, c.GO:c.GO + c.L] = x_shard
    xt = np.swapaxes(np.asarray(x_shard, np.float32), 1, 2)        # [B_SH, C, L]
    return {
        'xgbf': xgp.reshape(c.B_SH * (c.L + c.GR), c.C).astype(BF),
        'xtbf': np.ascontiguousarray(xt).reshape(c.B_SH * c.C, c.L).astype(BF),
    }


# ----------------------------------------------------------------------------
# Public entry point
# ----------------------------------------------------------------------------
_N_CORES = 8
_B, _L, _C, _H, _G, _K = 16, 2048, 1024, 16, 4, 5
_CACHE = {}


def _get_nc(cfg):
    if 'nc' not in _CACHE:
        _CACHE['nc'] = make_nc(cfg)
    return _CACHE['nc']


def kernel(**inputs):
    inputs = {k: np.asarray(v) for k, v in inputs.items()}
    cfg = Cfg(B_SH=_B // _N_CORES, L=_L, C=_C, H=_H, G=_G, K=_K)
    cfg.set_mm_gather(False)
    nc = _get_nc(cfg)
    sh = host_prep_shared(inputs, cfg)
    in_maps = [
        {**sh, **host_prep_core(inputs['x'][c * cfg.B_SH:(c + 1) * cfg.B_SH], cfg)}
        for c in range(_N_CORES)
    ]
    from concourse.bass_utils import run_bass_kernel_spmd
    res = run_bass_kernel_spmd(nc, in_maps, core_ids=list(range(_N_CORES)))
    out = np.concatenate(
        [res.results[c]["out2d"].reshape(cfg.B_SH, _L, _C) for c in range(_N_CORES)],
        axis=0)
    return out.astype(np.float32)
